# revision 20
# baseline (speedup 1.0000x reference)
"""BandSplitModule Trainium2 kernel (v3).

Math (per band k with c=2w channels, folding layernorm affine + linear):
  out[n,t] = invstd[t] * sum_c X[c,t]*W2[c,n] + v[n]
where
  W2[c,n] = g[c]*W[c,n] - mean_c'(g*W)[n]     (removes the mean term)
  v[n]    = sum_c b[c]*W[c,n] + cbias[n]
  invstd[t] = 1/sqrt(E[X^2] - E[X]^2)         (eps=1e-8 is far below
                                               bf16/f32 noise at var~1)
The invstd multiply is folded into the matmul by pre-scaling X columns.

Per core: one batch element. Bands are packed into 16 "super-tiles" of
128 partitions ([4 x c32] | [2 x c64] | [1 x c128]). X is cast to bf16
and pre-packed host-side into the exact SBUF layout ([128, 16*T]) so
each group loads with a few contiguous ~1MB DMAs. Consts are packed
into 4 tensors (4 DMAs). Column sums (stats) use ones-matmuls into
per-(group,chunk) PSUM tiles; X**2 runs on DVE in bf16 two chunks at a
time. Per-chunk var -> 1/var (DVE reciprocal) -> sqrt (Act, bf16 out)
keeps the invstd chain barrier-free across chunks; a bf16 selector
matmul broadcasts invstd rows to 128 partitions. Main matmuls are
bf16 with fp32 PSUM; [128,1024] 2-bank PSUM drains on Act fold the
bias. Output stores are one 1MB DMA per band.
"""
import numpy as np

B, F, T = 8, 1025, 2048
NF = 128                       # features
CHUNK = 512
NCH = T // CHUNK               # 4
NST = 16                       # total super-tiles

# (start_bin, width, n_bands) per group; c = 2*w channels per band
GROUP_DEFS = [(0, 16, 16), (256, 32, 8), (512, 64, 8)]

_cache = {}


def _supertiles():
    """Groups; each: dict(gi, c, w, K, s0 (first st), sts: list of super-
    tiles, each a list of (global_band, idx_in_group, part_off, row0))."""
    groups = []
    gb = 0
    st0_global = 0
    for gi, (s, w, nb) in enumerate(GROUP_DEFS):
        c = 2 * w
        per_st = 128 // c
        sts = []
        for st0 in range(0, nb, per_st):
            bands = []
            for j in range(per_st):
                bi = st0 + j
                bands.append((gb + bi, bi, j * c, s + bi * w))
            sts.append(bands)
        groups.append(dict(gi=gi, c=c, w=w, K=nb, sts=sts, s0=st0_global))
        st0_global += len(sts)
        gb += nb
    return groups


def _bf16():
    import ml_dtypes
    return np.dtype(ml_dtypes.bfloat16)


def _precompute(inputs):
    """Host-side folded weights / selectors / ones (float64 math)."""
    bf16 = _bf16()
    groups = _supertiles()
    w2 = np.zeros((128, NST * NF), np.float64)
    vmat = np.zeros((128, 32), np.float64)
    ones = np.zeros((128, NST * 2 * 64), bf16)
    sel = np.zeros((16, NST * NF), bf16)
    tags = ("16", "32", "64")
    for g in groups:
        gi, c, K, s0 = g["gi"], g["c"], g["K"], g["s0"]
        tag = tags[gi]
        gg = np.asarray(inputs["g" + tag], np.float64)
        bb = np.asarray(inputs["b" + tag], np.float64)
        WW = np.asarray(inputs["W" + tag], np.float64)
        cc = np.asarray(inputs["c" + tag], np.float64)
        for si, bands in enumerate(g["sts"]):
            st = s0 + si
            for (gband, ig, off, _r0) in bands:
                Wg = gg[ig][:, None] * WW[ig]            # (c, NF)
                W2b = Wg - Wg.mean(axis=0, keepdims=True)
                w2[off:off + c, st * NF:(st + 1) * NF] = W2b
                vmat[:, gband] = bb[ig] @ WW[ig] + cc[ig]
                ones[off:off + c, 2 * st * 64 + ig] = 1.0
                ones[off:off + c, (2 * st + 1) * 64 + 32 + ig] = 1.0
                sel[ig, st * NF + off:st * NF + off + c] = 1.0
    return dict(w2=w2.astype(bf16), vmat=vmat.astype(np.float32),
                ones=ones, sel=sel)


def _pack_x(x_real, x_imag):
    """[B,F,T] f32 pair -> [B, 128, 16*T] bf16 in super-tile SBUF layout."""
    bf16 = _bf16()
    xp = np.zeros((B, 128, NST, T), bf16)
    # group16: sts 0..3, 4 bands x c=32; band j -> partitions 32j(+16 imag)
    xr = x_real[:, 0:256, :].reshape(B, 4, 4, 16, T)    # (b, st, j, row, t)
    xi = x_imag[:, 0:256, :].reshape(B, 4, 4, 16, T)
    v = xp.reshape(B, 4, 32, NST, T)                    # (b, j, chan, st, t)
    v[:, :, 0:16, 0:4, :] = xr.transpose(0, 2, 3, 1, 4)
    v[:, :, 16:32, 0:4, :] = xi.transpose(0, 2, 3, 1, 4)
    # group32: sts 4..7, 2 bands x c=64
    xr = x_real[:, 256:512, :].reshape(B, 4, 2, 32, T)
    xi = x_imag[:, 256:512, :].reshape(B, 4, 2, 32, T)
    v = xp.reshape(B, 2, 64, NST, T)
    v[:, :, 0:32, 4:8, :] = xr.transpose(0, 2, 3, 1, 4)
    v[:, :, 32:64, 4:8, :] = xi.transpose(0, 2, 3, 1, 4)
    # group64: sts 8..15, 1 band x c=128
    xr = x_real[:, 512:1024, :].reshape(B, 8, 64, T)
    xi = x_imag[:, 512:1024, :].reshape(B, 8, 64, T)
    xp[:, 0:64, 8:16, :] = xr.transpose(0, 2, 1, 3)
    xp[:, 64:128, 8:16, :] = xi.transpose(0, 2, 1, 3)
    return np.ascontiguousarray(xp.reshape(B, 128, NST * T))


def _build_nc(reps=1, bench=False, mode="full", pipe=True):
    import concourse.bass as bass
    import concourse.tile as tile
    from concourse import mybir

    f32 = mybir.dt.float32
    bf16 = mybir.dt.bfloat16
    AF = mybir.ActivationFunctionType
    ALU = mybir.AluOpType

    groups = _supertiles()

    do_xdma = mode in ("full", "dma")
    do_out = mode in ("full", "dma")
    do_mm = mode in ("full", "compute", "mm")
    do_dve = mode in ("full", "compute", "dve")
    do_act = mode in ("full", "compute", "act")

    ikind = "Internal" if bench else "ExternalInput"
    okind = "Internal" if bench else "ExternalOutput"
    nc = bass.Bass("TRN2", debug=False)
    xpd = nc.dram_tensor("xpack", [128, NST * T], bf16, kind=ikind).ap()
    w2d = nc.dram_tensor("w2", [128, NST * NF], bf16, kind=ikind).ap()
    onesd = nc.dram_tensor("ones", [128, NST * 2 * 64], bf16, kind=ikind).ap()
    seld = nc.dram_tensor("sel", [16, NST * NF], bf16, kind=ikind).ap()
    vd = nc.dram_tensor("vmat", [128, 32], f32, kind=ikind).ap()
    outd = nc.dram_tensor("out", [128, 32, T], f32, kind=okind).ap()
    benchd = None
    if bench:
        benchd = nc.dram_tensor("bench", [128, 32], f32,
                                kind="ExternalOutput").ap()

    with tile.TileContext(nc) as tc:
        with tc.tile_pool(name="consts", bufs=2) as consts, \
             tc.tile_pool(name="xp", bufs=1) as xp, \
             tc.tile_pool(name="x2p", bufs=10) as x2p, \
             tc.tile_pool(name="cmp", bufs=1) as cmp_, \
             tc.tile_pool(name="outp", bufs=5) as outp, \
             tc.tile_pool(name="ps_stats", bufs=2, space="PSUM") as ps_stats, \
             tc.tile_pool(name="ps_a", bufs=2, space="PSUM") as ps_a, \
             tc.tile_pool(name="ps_main", bufs=2, space="PSUM") as ps_main:

            vt = None
            for _rep in range(reps):
                # ---- constants ----
                wt = consts.tile([128, NST * NF], bf16, tag="w2", name="wt")
                nc.sync.dma_start(out=wt[:], in_=w2d[:])
                onest = consts.tile([128, NST * 2 * 64], bf16, tag="ones",
                                    name="onest")
                nc.sync.dma_start(out=onest[:], in_=onesd[:])
                selt = consts.tile([16, NST * NF], bf16, tag="sel",
                                   name="selt")
                nc.sync.dma_start(out=selt[:], in_=seld[:])
                vt = consts.tile([128, 32], f32, tag="vmat", name="vt")
                nc.sync.dma_start(out=vt[:], in_=vd[:])

                # ---- per group, software-pipelined ----
                # Phase A (stats+invstd) of group g is emitted before
                # phase B (scale/project/store) of group g-1, so the
                # in-order engines always have independent work to hide
                # the cross-engine invstd chain latency.
                def phase_a(g):
                    gi, c, K, s0 = g["gi"], g["c"], g["K"], g["s0"]
                    sts = g["sts"]
                    nst = len(sts)
                    inv_c = 1.0 / c

                    xt = xp.tile([128, nst * T], bf16, tag=f"X{gi}",
                                 name=f"xt{gi}", bufs=(1 if gi < 2 else 2))
                    if do_xdma:
                        for p0 in range(0, nst, 2):
                            p1 = min(p0 + 2, nst)
                            nc.sync.dma_start(
                                out=xt[:, p0 * T:p1 * T],
                                in_=xpd[:, (s0 + p0) * T:(s0 + p1) * T])
                    else:
                        nc.vector.memset(xt[:, 0:1], 0.0)

                    if mode == "dma":
                        for si, bands in enumerate(sts):
                            for (gband, _ig, off, _r0) in bands:
                                ot = outp.tile([128, T], f32, tag="O",
                                               name="ot")
                                nc.vector.memset(ot[:, 0:1], 0.0)
                                nc.sync.dma_start(out=outd[:, gband, :],
                                                  in_=ot[:])
                        return xt, None

                    # stats + per-chunk invstd (no group-wide barrier).
                    # varrb/rv/arbh are free-dim chunked: [K, ch*512 ...].
                    varrb = cmp_.tile([16, NCH * CHUNK], f32, tag="varrb",
                                      name="varrb")
                    rv = cmp_.tile([16, NCH * CHUNK], f32, tag="rv",
                                   name="rv")
                    arbh = cmp_.tile([16, NCH * CHUNK], bf16, tag="arbh",
                                     name="arbh", bufs=2)
                    if not do_dve:
                        nc.vector.memset(varrb[:, 0:1], 1.0)
                        nc.vector.memset(rv[:, 0:1], 1.0)
                    if not do_act:
                        nc.vector.memset(arbh[:, 0:1], 1.0)
                    sqs = {}
                    for ch in range(NCH):
                        cs = slice(ch * CHUNK, (ch + 1) * CHUNK)
                        stats = ps_stats.tile([64, CHUNK], f32, tag="stats",
                                              name="stats")
                        for si in range(nst):
                            st = s0 + si
                            xs = xt[:, si * T + ch * CHUNK:
                                    si * T + (ch + 1) * CHUNK]
                            if ch % 2 == 0:
                                sq = x2p.tile([128, 2 * CHUNK], bf16,
                                              tag="sq", name="sq")
                                sqs[si] = sq
                                if do_dve:
                                    xs2 = xt[:, si * T + ch * CHUNK:
                                             si * T + (ch + 2) * CHUNK]
                                    nc.vector.tensor_mul(sq[:], xs2, xs2)
                                else:
                                    nc.vector.memset(sq[:, 0:1], 0.0)
                            sqv = sqs[si][:, (ch % 2) * CHUNK:
                                          (ch % 2 + 1) * CHUNK]
                            if do_mm:
                                nc.tensor.matmul(
                                    stats[:],
                                    onest[:, 2 * st * 64:(2 * st + 1) * 64],
                                    xs, start=(si == 0), stop=False,
                                    skip_group_check=True)
                                nc.tensor.matmul(
                                    stats[:],
                                    onest[:, (2 * st + 1) * 64:
                                          (2 * st + 2) * 64],
                                    sqv, start=False, stop=(si == nst - 1),
                                    skip_group_check=True)
                        if not do_mm:
                            nc.vector.memset(stats[:, 0:1], 1.0)
                        m2 = cmp_.tile([16, CHUNK], f32, tag="m2", name="m2")
                        if do_act:
                            nc.scalar.activation(m2[0:K, :], stats[0:K, :],
                                                 AF.Square, scale=inv_c)
                        else:
                            nc.vector.memset(m2[:, 0:1], 0.0)
                        if do_dve:
                            # var = E[X^2] - E[X]^2, then 1/var
                            nc.vector.scalar_tensor_tensor(
                                varrb[0:K, cs], stats[32:32 + K, :],
                                inv_c, m2[0:K, :], ALU.mult, ALU.subtract)
                            nc.vector.reciprocal(rv[0:K, cs], varrb[0:K, cs])
                        if do_act:
                            # invstd = sqrt(1/var), rounded to bf16
                            nc.scalar.activation(arbh[0:K, cs], rv[0:K, cs],
                                                 AF.Sqrt)
                    return xt, arbh

                def phase_b(g, xt, arbh):
                    gi, c, K, s0 = g["gi"], g["c"], g["K"], g["s0"]
                    sts = g["sts"]
                    for si, bands in enumerate(sts):
                        st = s0 + si
                        for ch in range(NCH):
                            at = ps_a.tile([128, CHUNK], f32, tag="at",
                                           name="at")
                            if do_mm:
                                nc.tensor.matmul(
                                    at[:],
                                    selt[0:K, st * NF:(st + 1) * NF],
                                    arbh[0:K, ch * CHUNK:(ch + 1) * CHUNK],
                                    start=True, stop=True)
                            elif do_dve:
                                nc.vector.memset(at[:, 0:1], 1.0)
                            xs = xt[:, si * T + ch * CHUNK:
                                    si * T + (ch + 1) * CHUNK]
                            if do_dve:
                                nc.vector.tensor_mul(xs, xs, at[:])
                        for (gband, _ig, off, _r0) in bands:
                            ot = outp.tile([128, T], f32, tag="O", name="ot")
                            for h in range(2):
                                pm = ps_main.tile([128, 2 * CHUNK], f32,
                                                  tag="pm", name="pm")
                                if do_mm:
                                    for cc in range(2):
                                        ch = 2 * h + cc
                                        nc.tensor.matmul(
                                            pm[:, cc * CHUNK:(cc + 1) * CHUNK],
                                            wt[off:off + c,
                                               st * NF:(st + 1) * NF],
                                            xt[off:off + c,
                                               si * T + ch * CHUNK:
                                               si * T + (ch + 1) * CHUNK],
                                            start=True, stop=True,
                                            skip_group_check=True,
                                            tile_position=(off, 0))
                                else:
                                    nc.vector.memset(pm[:, 0:1], 0.0)
                                if do_act:
                                    nc.scalar.activation(
                                        ot[:, h * 2 * CHUNK:
                                           (h + 1) * 2 * CHUNK],
                                        pm[:], AF.Identity,
                                        bias=vt[:, gband:gband + 1])
                                elif not do_out:
                                    nc.vector.memset(ot[:, 0:1], 0.0)
                            if do_out:
                                nc.sync.dma_start(out=outd[:, gband, :],
                                                  in_=ot[:])

                pending = None
                for g in groups:
                    res = phase_a(g)
                    if mode == "dma":
                        continue
                    if not pipe:
                        phase_b(g, res[0], res[1])
                        continue
                    if pending is not None:
                        phase_b(*pending)
                    pending = (g, res[0], res[1])
                if pending is not None and mode != "dma":
                    phase_b(*pending)
            if bench and benchd is not None and vt is not None:
                nc.sync.dma_start(out=benchd[:], in_=vt[:])
    return nc


def _split_excess_waits(nc, max_waits=1):
    """This walrus build rejects >1 semaphore wait on compute-instruction
    templates, while Tile freely attaches several. Hoist all but one wait
    onto standalone InstEventSemaphore instructions inserted just before,
    on the same engine — semantically identical (AND of ge-waits, engine
    stalls in program order)."""
    import concourse.mybir as mybir

    counter = 0
    for f in nc.m.functions:
        for blk in f.blocks:
            new_list = []
            changed = False
            for ins in blk.instructions:
                si = ins.sync_info
                ow = list(si.on_wait) if si is not None and si.on_wait else []
                if (
                    len(ow) > max_waits
                    and type(ins).__name__ != "InstEventSemaphore"
                    and all(w.wait_mode == "sem-ge-imm" for w in ow)
                ):
                    for w in ow[:-max_waits]:
                        ev = mybir.InstEventSemaphore(
                            name=f"evwait_split_{counter}", ins=[], outs=[]
                        )
                        counter += 1
                        ev.engine = ins.engine
                        ev.bass_nofuse = True
                        ev.debug = ins.debug
                        ev.sync_info = mybir.SyncInfo(on_wait=[w], on_update=[])
                        new_list.append(ev)
                    ins.sync_info = mybir.SyncInfo(
                        on_wait=ow[-max_waits:],
                        on_update=list(si.on_update) if si.on_update else [],
                    )
                    changed = True
                new_list.append(ins)
            if changed:
                blk.instructions = new_list
    return counter


def _get_nc(reps=1, bench=False, mode="full", pipe=True):
    key = f"nc{reps}_{bench}_{mode}_{pipe}"
    if key not in _cache:
        nc = _build_nc(reps, bench, mode, pipe)
        _split_excess_waits(nc)
        _cache[key] = nc
    return _cache[key]


def _get_bench_nc_nopipe(reps):
    return _get_nc(reps, bench=True, pipe=False)


def _get_bench_nc(reps):
    return _get_nc(reps, bench=True)


def _get_bench_nc_dma(reps):
    return _get_nc(reps, bench=True, mode="dma")


def _get_bench_nc_compute(reps):
    return _get_nc(reps, bench=True, mode="compute")


def _bench_mode(mode):
    return lambda reps: _get_nc(reps, bench=True, mode=mode)


def make_imap(inputs):
    """Returns imap(core)->input dict, for the test harness's timing path."""
    consts = _precompute(inputs)
    xpack = _pack_x(np.asarray(inputs["x_real"], np.float32),
                    np.asarray(inputs["x_imag"], np.float32))

    def imap(b):
        return {
            "xpack": xpack[b], "w2": consts["w2"], "ones": consts["ones"],
            "sel": consts["sel"], "vmat": consts["vmat"],
        }
    return imap


def kernel(**inputs):
    from concourse.bass_utils import run_bass_kernel_spmd

    imap = make_imap(inputs)
    in_maps = [imap(b) for b in range(B)]
    nc = _get_nc()
    res = run_bass_kernel_spmd(nc, in_maps, list(range(B)))
    out = np.stack([res.results[b]["out"] for b in range(B)], axis=0)
    return out


# revision 21
# speedup vs baseline: 1.0615x; 1.0615x over previous
"""BandSplitModule Trainium2 kernel (v3).

Math (per band k with c=2w channels, folding layernorm affine + linear):
  out[n,t] = invstd[t] * sum_c X[c,t]*W2[c,n] + v[n]
where
  W2[c,n] = g[c]*W[c,n] - mean_c'(g*W)[n]     (removes the mean term)
  v[n]    = sum_c b[c]*W[c,n] + cbias[n]
  invstd[t] = 1/sqrt(E[X^2] - E[X]^2)         (eps=1e-8 is far below
                                               bf16/f32 noise at var~1)
The invstd multiply is folded into the matmul by pre-scaling X columns.

Per core: one batch element. Bands are packed into 16 "super-tiles" of
128 partitions ([4 x c32] | [2 x c64] | [1 x c128]). X is cast to bf16
and pre-packed host-side into the exact SBUF layout ([128, 16*T]) so
each group loads with a few contiguous ~1MB DMAs. Consts are packed
into 4 tensors (4 DMAs). Column sums (stats) use ones-matmuls into
per-(group,chunk) PSUM tiles; X**2 runs on DVE in bf16 two chunks at a
time. Per-chunk var -> 1/var (DVE reciprocal) -> sqrt (Act, bf16 out)
keeps the invstd chain barrier-free across chunks; a bf16 selector
matmul broadcasts invstd rows to 128 partitions. Main matmuls are
bf16 with fp32 PSUM; [128,1024] 2-bank PSUM drains on Act fold the
bias. Output stores are one 1MB DMA per band.
"""
import numpy as np

B, F, T = 8, 1025, 2048
NF = 128                       # features
CHUNK = 512
NCH = T // CHUNK               # 4
NST = 16                       # total super-tiles

# (start_bin, width, n_bands) per group; c = 2*w channels per band
GROUP_DEFS = [(0, 16, 16), (256, 32, 8), (512, 64, 8)]

_cache = {}


def _supertiles():
    """Groups; each: dict(gi, c, w, K, s0 (first st), sts: list of super-
    tiles, each a list of (global_band, idx_in_group, part_off, row0))."""
    groups = []
    gb = 0
    st0_global = 0
    for gi, (s, w, nb) in enumerate(GROUP_DEFS):
        c = 2 * w
        per_st = 128 // c
        sts = []
        for st0 in range(0, nb, per_st):
            bands = []
            for j in range(per_st):
                bi = st0 + j
                bands.append((gb + bi, bi, j * c, s + bi * w))
            sts.append(bands)
        groups.append(dict(gi=gi, c=c, w=w, K=nb, sts=sts, s0=st0_global))
        st0_global += len(sts)
        gb += nb
    return groups


def _bf16():
    import ml_dtypes
    return np.dtype(ml_dtypes.bfloat16)


def _precompute(inputs):
    """Host-side folded weights / selectors / ones (float64 math)."""
    bf16 = _bf16()
    groups = _supertiles()
    w2 = np.zeros((128, NST * NF), np.float64)
    vmat = np.zeros((128, 32), np.float64)
    ones = np.zeros((128, NST * 2 * 64), bf16)
    sel = np.zeros((16, NST * NF), bf16)
    tags = ("16", "32", "64")
    for g in groups:
        gi, c, K, s0 = g["gi"], g["c"], g["K"], g["s0"]
        tag = tags[gi]
        gg = np.asarray(inputs["g" + tag], np.float64)
        bb = np.asarray(inputs["b" + tag], np.float64)
        WW = np.asarray(inputs["W" + tag], np.float64)
        cc = np.asarray(inputs["c" + tag], np.float64)
        for si, bands in enumerate(g["sts"]):
            st = s0 + si
            for (gband, ig, off, _r0) in bands:
                Wg = gg[ig][:, None] * WW[ig]            # (c, NF)
                W2b = Wg - Wg.mean(axis=0, keepdims=True)
                w2[off:off + c, st * NF:(st + 1) * NF] = W2b
                vmat[:, gband] = bb[ig] @ WW[ig] + cc[ig]
                ones[off:off + c, 2 * st * 64 + ig] = 1.0
                ones[off:off + c, (2 * st + 1) * 64 + 32 + ig] = 1.0
                sel[ig, st * NF + off:st * NF + off + c] = 1.0
    return dict(w2=w2.astype(bf16), vmat=vmat.astype(np.float32),
                ones=ones, sel=sel)


def _pack_x(x_real, x_imag):
    """[B,F,T] f32 pair -> [B, 128, 16*T] bf16 in super-tile SBUF layout."""
    bf16 = _bf16()
    xp = np.zeros((B, 128, NST, T), bf16)
    # group16: sts 0..3, 4 bands x c=32; band j -> partitions 32j(+16 imag)
    xr = x_real[:, 0:256, :].reshape(B, 4, 4, 16, T)    # (b, st, j, row, t)
    xi = x_imag[:, 0:256, :].reshape(B, 4, 4, 16, T)
    v = xp.reshape(B, 4, 32, NST, T)                    # (b, j, chan, st, t)
    v[:, :, 0:16, 0:4, :] = xr.transpose(0, 2, 3, 1, 4)
    v[:, :, 16:32, 0:4, :] = xi.transpose(0, 2, 3, 1, 4)
    # group32: sts 4..7, 2 bands x c=64
    xr = x_real[:, 256:512, :].reshape(B, 4, 2, 32, T)
    xi = x_imag[:, 256:512, :].reshape(B, 4, 2, 32, T)
    v = xp.reshape(B, 2, 64, NST, T)
    v[:, :, 0:32, 4:8, :] = xr.transpose(0, 2, 3, 1, 4)
    v[:, :, 32:64, 4:8, :] = xi.transpose(0, 2, 3, 1, 4)
    # group64: sts 8..15, 1 band x c=128
    xr = x_real[:, 512:1024, :].reshape(B, 8, 64, T)
    xi = x_imag[:, 512:1024, :].reshape(B, 8, 64, T)
    xp[:, 0:64, 8:16, :] = xr.transpose(0, 2, 1, 3)
    xp[:, 64:128, 8:16, :] = xi.transpose(0, 2, 1, 3)
    return np.ascontiguousarray(xp.reshape(B, 128, NST * T))


def _build_nc(reps=1, bench=False, mode="full", pipe=True, fulldrain=False):
    import concourse.bass as bass
    import concourse.tile as tile
    from concourse import mybir

    f32 = mybir.dt.float32
    bf16 = mybir.dt.bfloat16
    AF = mybir.ActivationFunctionType
    ALU = mybir.AluOpType

    groups = _supertiles()

    do_xdma = mode in ("full", "dma")
    do_out = mode in ("full", "dma")
    do_mm = mode in ("full", "compute", "mm")
    do_dve = mode in ("full", "compute", "dve")
    do_act = mode in ("full", "compute", "act")

    ikind = "Internal" if bench else "ExternalInput"
    okind = "Internal" if bench else "ExternalOutput"
    nc = bass.Bass("TRN2", debug=False)
    xpd = nc.dram_tensor("xpack", [128, NST * T], bf16, kind=ikind).ap()
    w2d = nc.dram_tensor("w2", [128, NST * NF], bf16, kind=ikind).ap()
    onesd = nc.dram_tensor("ones", [128, NST * 2 * 64], bf16, kind=ikind).ap()
    seld = nc.dram_tensor("sel", [16, NST * NF], bf16, kind=ikind).ap()
    vd = nc.dram_tensor("vmat", [128, 32], f32, kind=ikind).ap()
    outd = nc.dram_tensor("out", [128, 32, T], f32, kind=okind).ap()
    benchd = None
    if bench:
        benchd = nc.dram_tensor("bench", [128, 32], f32,
                                kind="ExternalOutput").ap()

    with tile.TileContext(nc) as tc:
        with tc.tile_pool(name="consts", bufs=2) as consts, \
             tc.tile_pool(name="xp", bufs=1) as xp, \
             tc.tile_pool(name="x2p", bufs=10) as x2p, \
             tc.tile_pool(name="cmp", bufs=1) as cmp_, \
             tc.tile_pool(name="outp", bufs=5) as outp, \
             tc.tile_pool(name="ps_stats", bufs=2, space="PSUM") as ps_stats, \
             tc.tile_pool(name="ps_a", bufs=2, space="PSUM") as ps_a, \
             tc.tile_pool(name="ps_main", bufs=2, space="PSUM") as ps_main:

            vt = None
            for _rep in range(reps):
                # ---- constants ----
                wt = consts.tile([128, NST * NF], bf16, tag="w2", name="wt")
                nc.sync.dma_start(out=wt[:], in_=w2d[:])
                onest = consts.tile([128, NST * 2 * 64], bf16, tag="ones",
                                    name="onest")
                nc.sync.dma_start(out=onest[:], in_=onesd[:])
                selt = consts.tile([16, NST * NF], bf16, tag="sel",
                                   name="selt")
                nc.sync.dma_start(out=selt[:], in_=seld[:])
                vt = consts.tile([128, 32], f32, tag="vmat", name="vt")
                nc.sync.dma_start(out=vt[:], in_=vd[:])

                # ---- per group, software-pipelined ----
                # Phase A (stats+invstd) of group g is emitted before
                # phase B (scale/project/store) of group g-1, so the
                # in-order engines always have independent work to hide
                # the cross-engine invstd chain latency.
                def phase_a(g):
                    gi, c, K, s0 = g["gi"], g["c"], g["K"], g["s0"]
                    sts = g["sts"]
                    nst = len(sts)
                    inv_c = 1.0 / c

                    xt = xp.tile([128, nst * T], bf16, tag=f"X{gi}",
                                 name=f"xt{gi}", bufs=(1 if gi < 2 else 2))
                    if do_xdma:
                        for p0 in range(0, nst, 2):
                            p1 = min(p0 + 2, nst)
                            nc.sync.dma_start(
                                out=xt[:, p0 * T:p1 * T],
                                in_=xpd[:, (s0 + p0) * T:(s0 + p1) * T])
                    else:
                        nc.vector.memset(xt[:, 0:1], 0.0)

                    if mode == "dma":
                        for si, bands in enumerate(sts):
                            for (gband, _ig, off, _r0) in bands:
                                ot = outp.tile([128, T], f32, tag="O",
                                               name="ot")
                                nc.vector.memset(ot[:, 0:1], 0.0)
                                nc.sync.dma_start(out=outd[:, gband, :],
                                                  in_=ot[:])
                        return xt, None

                    # stats + per-chunk invstd (no group-wide barrier).
                    # varrb/rv/arbh are free-dim chunked: [K, ch*512 ...].
                    varrb = cmp_.tile([16, NCH * CHUNK], f32, tag="varrb",
                                      name="varrb")
                    rv = cmp_.tile([16, NCH * CHUNK], f32, tag="rv",
                                   name="rv")
                    arbh = cmp_.tile([16, NCH * CHUNK], bf16, tag="arbh",
                                     name="arbh", bufs=2)
                    if not do_dve:
                        nc.vector.memset(varrb[:, 0:1], 1.0)
                        nc.vector.memset(rv[:, 0:1], 1.0)
                    if not do_act:
                        nc.vector.memset(arbh[:, 0:1], 1.0)
                    sqs = {}
                    for ch in range(NCH):
                        cs = slice(ch * CHUNK, (ch + 1) * CHUNK)
                        stats = ps_stats.tile([64, CHUNK], f32, tag="stats",
                                              name="stats")
                        for si in range(nst):
                            st = s0 + si
                            xs = xt[:, si * T + ch * CHUNK:
                                    si * T + (ch + 1) * CHUNK]
                            if ch % 2 == 0:
                                sq = x2p.tile([128, 2 * CHUNK], bf16,
                                              tag="sq", name="sq")
                                sqs[si] = sq
                                if do_dve:
                                    xs2 = xt[:, si * T + ch * CHUNK:
                                             si * T + (ch + 2) * CHUNK]
                                    nc.vector.tensor_mul(sq[:], xs2, xs2)
                                else:
                                    nc.vector.memset(sq[:, 0:1], 0.0)
                            sqv = sqs[si][:, (ch % 2) * CHUNK:
                                          (ch % 2 + 1) * CHUNK]
                            if do_mm:
                                nc.tensor.matmul(
                                    stats[:],
                                    onest[:, 2 * st * 64:(2 * st + 1) * 64],
                                    xs, start=(si == 0), stop=False,
                                    skip_group_check=True)
                                nc.tensor.matmul(
                                    stats[:],
                                    onest[:, (2 * st + 1) * 64:
                                          (2 * st + 2) * 64],
                                    sqv, start=False, stop=(si == nst - 1),
                                    skip_group_check=True)
                        if not do_mm:
                            nc.vector.memset(stats[:, 0:1], 1.0)
                        m2 = cmp_.tile([16, CHUNK], f32, tag="m2", name="m2")
                        if do_act:
                            nc.scalar.activation(m2[0:K, :], stats[0:K, :],
                                                 AF.Square, scale=inv_c)
                        else:
                            nc.vector.memset(m2[:, 0:1], 0.0)
                        if do_dve:
                            # var = E[X^2] - E[X]^2, then 1/var
                            nc.vector.scalar_tensor_tensor(
                                varrb[0:K, cs], stats[32:32 + K, :],
                                inv_c, m2[0:K, :], ALU.mult, ALU.subtract)
                            nc.vector.reciprocal(rv[0:K, cs], varrb[0:K, cs])
                        if do_act:
                            # invstd = sqrt(1/var), rounded to bf16
                            nc.scalar.activation(arbh[0:K, cs], rv[0:K, cs],
                                                 AF.Sqrt)
                    return xt, arbh

                def phase_b(g, xt, arbh):
                    gi, c, K, s0 = g["gi"], g["c"], g["K"], g["s0"]
                    sts = g["sts"]
                    for si, bands in enumerate(sts):
                        st = s0 + si
                        for ch in range(NCH):
                            at = ps_a.tile([128, CHUNK], f32, tag="at",
                                           name="at")
                            if do_mm:
                                nc.tensor.matmul(
                                    at[:],
                                    selt[0:K, st * NF:(st + 1) * NF],
                                    arbh[0:K, ch * CHUNK:(ch + 1) * CHUNK],
                                    start=True, stop=True)
                            elif do_dve:
                                nc.vector.memset(at[:, 0:1], 1.0)
                            xs = xt[:, si * T + ch * CHUNK:
                                    si * T + (ch + 1) * CHUNK]
                            if do_dve:
                                nc.vector.tensor_mul(xs, xs, at[:])
                        for (gband, _ig, off, _r0) in bands:
                            ot = outp.tile([128, T], f32, tag="O", name="ot")
                            nh, hw_ = (1, 4) if fulldrain else (2, 2)
                            for h in range(nh):
                                pm = ps_main.tile(
                                    [128, hw_ * CHUNK], f32, tag="pm",
                                    name="pm", bufs=(1 if fulldrain else 2))
                                if do_mm:
                                    for cc in range(hw_):
                                        ch = hw_ * h + cc
                                        nc.tensor.matmul(
                                            pm[:, cc * CHUNK:(cc + 1) * CHUNK],
                                            wt[off:off + c,
                                               st * NF:(st + 1) * NF],
                                            xt[off:off + c,
                                               si * T + ch * CHUNK:
                                               si * T + (ch + 1) * CHUNK],
                                            start=True, stop=True,
                                            skip_group_check=True,
                                            tile_position=(off, 0))
                                else:
                                    nc.vector.memset(pm[:, 0:1], 0.0)
                                if do_act:
                                    nc.scalar.activation(
                                        ot[:, h * hw_ * CHUNK:
                                           (h + 1) * hw_ * CHUNK],
                                        pm[:], AF.Identity,
                                        bias=vt[:, gband:gband + 1])
                                elif not do_out:
                                    nc.vector.memset(ot[:, 0:1], 0.0)
                            if do_out:
                                nc.sync.dma_start(out=outd[:, gband, :],
                                                  in_=ot[:])

                pending = None
                for g in groups:
                    res = phase_a(g)
                    if mode == "dma":
                        continue
                    if not pipe:
                        phase_b(g, res[0], res[1])
                        continue
                    if pending is not None:
                        phase_b(*pending)
                    pending = (g, res[0], res[1])
                if pending is not None and mode != "dma":
                    phase_b(*pending)
            if bench and benchd is not None and vt is not None:
                nc.sync.dma_start(out=benchd[:], in_=vt[:])
    return nc


def _split_excess_waits(nc, max_waits=1):
    """This walrus build rejects >1 semaphore wait on compute-instruction
    templates, while Tile freely attaches several. Hoist all but one wait
    onto standalone InstEventSemaphore instructions inserted just before,
    on the same engine — semantically identical (AND of ge-waits, engine
    stalls in program order)."""
    import concourse.mybir as mybir

    counter = 0
    for f in nc.m.functions:
        for blk in f.blocks:
            new_list = []
            changed = False
            for ins in blk.instructions:
                si = ins.sync_info
                ow = list(si.on_wait) if si is not None and si.on_wait else []
                if (
                    len(ow) > max_waits
                    and type(ins).__name__ != "InstEventSemaphore"
                    and all(w.wait_mode == "sem-ge-imm" for w in ow)
                ):
                    for w in ow[:-max_waits]:
                        ev = mybir.InstEventSemaphore(
                            name=f"evwait_split_{counter}", ins=[], outs=[]
                        )
                        counter += 1
                        ev.engine = ins.engine
                        ev.bass_nofuse = True
                        ev.debug = ins.debug
                        ev.sync_info = mybir.SyncInfo(on_wait=[w], on_update=[])
                        new_list.append(ev)
                    ins.sync_info = mybir.SyncInfo(
                        on_wait=ow[-max_waits:],
                        on_update=list(si.on_update) if si.on_update else [],
                    )
                    changed = True
                new_list.append(ins)
            if changed:
                blk.instructions = new_list
    return counter


def _get_nc(reps=1, bench=False, mode="full", pipe=True, fulldrain=False):
    key = f"nc{reps}_{bench}_{mode}_{pipe}_{fulldrain}"
    if key not in _cache:
        nc = _build_nc(reps, bench, mode, pipe, fulldrain)
        _split_excess_waits(nc)
        _cache[key] = nc
    return _cache[key]


def _get_bench_nc_fulldrain(reps):
    return _get_nc(reps, bench=True, fulldrain=True)


def _get_bench_nc_nopipe(reps):
    return _get_nc(reps, bench=True, pipe=False)


def _get_bench_nc(reps):
    return _get_nc(reps, bench=True)


def _get_bench_nc_dma(reps):
    return _get_nc(reps, bench=True, mode="dma")


def _get_bench_nc_compute(reps):
    return _get_nc(reps, bench=True, mode="compute")


def _bench_mode(mode):
    return lambda reps: _get_nc(reps, bench=True, mode=mode)


def make_imap(inputs):
    """Returns imap(core)->input dict, for the test harness's timing path."""
    consts = _precompute(inputs)
    xpack = _pack_x(np.asarray(inputs["x_real"], np.float32),
                    np.asarray(inputs["x_imag"], np.float32))

    def imap(b):
        return {
            "xpack": xpack[b], "w2": consts["w2"], "ones": consts["ones"],
            "sel": consts["sel"], "vmat": consts["vmat"],
        }
    return imap


def kernel(**inputs):
    from concourse.bass_utils import run_bass_kernel_spmd

    imap = make_imap(inputs)
    in_maps = [imap(b) for b in range(B)]
    nc = _get_nc()
    res = run_bass_kernel_spmd(nc, in_maps, list(range(B)))
    out = np.stack([res.results[b]["out"] for b in range(B)], axis=0)
    return out


# revision 25
# speedup vs baseline: 1.0644x; 1.0028x over previous
"""BandSplitModule Trainium2 kernel (v3).

Math (per band k with c=2w channels, folding layernorm affine + linear):
  out[n,t] = invstd[t] * sum_c X[c,t]*W2[c,n] + v[n]
where
  W2[c,n] = g[c]*W[c,n] - mean_c'(g*W)[n]     (removes the mean term)
  v[n]    = sum_c b[c]*W[c,n] + cbias[n]
  invstd[t] = 1/sqrt(E[X^2] - E[X]^2)         (eps=1e-8 is far below
                                               bf16/f32 noise at var~1)
The invstd multiply is folded into the matmul by pre-scaling X columns.

Per core: one batch element. Bands are packed into 16 "super-tiles" of
128 partitions ([4 x c32] | [2 x c64] | [1 x c128]). X is cast to bf16
and pre-packed host-side into the exact SBUF layout ([128, 16*T]) so
each group loads with a few contiguous ~1MB DMAs. Consts are packed
into 4 tensors (4 DMAs). Column sums (stats) use ones-matmuls into
per-(group,chunk) PSUM tiles; X**2 runs on DVE in bf16 two chunks at a
time. Per-chunk var -> 1/var (DVE reciprocal) -> sqrt (Act, bf16 out)
keeps the invstd chain barrier-free across chunks; a bf16 selector
matmul broadcasts invstd rows to 128 partitions. Main matmuls are
bf16 with fp32 PSUM; [128,1024] 2-bank PSUM drains on Act fold the
bias. Output stores are one 1MB DMA per band.
"""
import numpy as np

B, F, T = 8, 1025, 2048
NF = 128                       # features
CHUNK = 512
NCH = T // CHUNK               # 4
NST = 16                       # total super-tiles

# (start_bin, width, n_bands) per group; c = 2*w channels per band
GROUP_DEFS = [(0, 16, 16), (256, 32, 8), (512, 64, 8)]

_cache = {}


def _supertiles():
    """Groups; each: dict(gi, c, w, K, s0 (first st), sts: list of super-
    tiles, each a list of (global_band, idx_in_group, part_off, row0))."""
    groups = []
    gb = 0
    st0_global = 0
    for gi, (s, w, nb) in enumerate(GROUP_DEFS):
        c = 2 * w
        per_st = 128 // c
        sts = []
        for st0 in range(0, nb, per_st):
            bands = []
            for j in range(per_st):
                bi = st0 + j
                bands.append((gb + bi, bi, j * c, s + bi * w))
            sts.append(bands)
        groups.append(dict(gi=gi, c=c, w=w, K=nb, sts=sts, s0=st0_global))
        st0_global += len(sts)
        gb += nb
    return groups


def _bf16():
    import ml_dtypes
    return np.dtype(ml_dtypes.bfloat16)


def _precompute(inputs):
    """Host-side folded weights / selectors / ones (float64 math)."""
    bf16 = _bf16()
    groups = _supertiles()
    w2 = np.zeros((128, NST * NF), np.float64)
    vmat = np.zeros((128, 32), np.float64)
    ones = np.zeros((128, NST * 2 * 64), bf16)
    sel = np.zeros((16, NST * NF), bf16)
    tags = ("16", "32", "64")
    for g in groups:
        gi, c, K, s0 = g["gi"], g["c"], g["K"], g["s0"]
        tag = tags[gi]
        gg = np.asarray(inputs["g" + tag], np.float64)
        bb = np.asarray(inputs["b" + tag], np.float64)
        WW = np.asarray(inputs["W" + tag], np.float64)
        cc = np.asarray(inputs["c" + tag], np.float64)
        for si, bands in enumerate(g["sts"]):
            st = s0 + si
            for (gband, ig, off, _r0) in bands:
                Wg = gg[ig][:, None] * WW[ig]            # (c, NF)
                W2b = Wg - Wg.mean(axis=0, keepdims=True)
                w2[off:off + c, st * NF:(st + 1) * NF] = W2b
                vmat[:, gband] = bb[ig] @ WW[ig] + cc[ig]
                ones[off:off + c, 2 * st * 64 + ig] = 1.0
                ones[off:off + c, (2 * st + 1) * 64 + 32 + ig] = 1.0
                sel[ig, st * NF + off:st * NF + off + c] = 1.0
    return dict(w2=w2.astype(bf16), vmat=vmat.astype(np.float32),
                ones=ones, sel=sel)


def _pack_x(x_real, x_imag):
    """[B,F,T] f32 pair -> [B, 128, 16*T] bf16 in super-tile SBUF layout."""
    bf16 = _bf16()
    xp = np.zeros((B, 128, NST, T), bf16)
    # group16: sts 0..3, 4 bands x c=32; band j -> partitions 32j(+16 imag)
    xr = x_real[:, 0:256, :].reshape(B, 4, 4, 16, T)    # (b, st, j, row, t)
    xi = x_imag[:, 0:256, :].reshape(B, 4, 4, 16, T)
    v = xp.reshape(B, 4, 32, NST, T)                    # (b, j, chan, st, t)
    v[:, :, 0:16, 0:4, :] = xr.transpose(0, 2, 3, 1, 4)
    v[:, :, 16:32, 0:4, :] = xi.transpose(0, 2, 3, 1, 4)
    # group32: sts 4..7, 2 bands x c=64
    xr = x_real[:, 256:512, :].reshape(B, 4, 2, 32, T)
    xi = x_imag[:, 256:512, :].reshape(B, 4, 2, 32, T)
    v = xp.reshape(B, 2, 64, NST, T)
    v[:, :, 0:32, 4:8, :] = xr.transpose(0, 2, 3, 1, 4)
    v[:, :, 32:64, 4:8, :] = xi.transpose(0, 2, 3, 1, 4)
    # group64: sts 8..15, 1 band x c=128
    xr = x_real[:, 512:1024, :].reshape(B, 8, 64, T)
    xi = x_imag[:, 512:1024, :].reshape(B, 8, 64, T)
    xp[:, 0:64, 8:16, :] = xr.transpose(0, 2, 1, 3)
    xp[:, 64:128, 8:16, :] = xi.transpose(0, 2, 1, 3)
    return np.ascontiguousarray(xp.reshape(B, 128, NST * T))


def _build_nc(reps=1, bench=False, mode="full", pipe=True, fulldrain=False,
              atpair=True, sqpool=False):
    import concourse.bass as bass
    import concourse.tile as tile
    from concourse import mybir

    f32 = mybir.dt.float32
    bf16 = mybir.dt.bfloat16
    AF = mybir.ActivationFunctionType
    ALU = mybir.AluOpType

    groups = _supertiles()

    do_xdma = mode in ("full", "dma")
    do_out = mode in ("full", "dma")
    do_mm = mode in ("full", "compute", "mm")
    do_dve = mode in ("full", "compute", "dve")
    do_act = mode in ("full", "compute", "act")

    ikind = "Internal" if bench else "ExternalInput"
    okind = "Internal" if bench else "ExternalOutput"
    nc = bass.Bass("TRN2", debug=False)
    xpd = nc.dram_tensor("xpack", [128, NST * T], bf16, kind=ikind).ap()
    w2d = nc.dram_tensor("w2", [128, NST * NF], bf16, kind=ikind).ap()
    onesd = nc.dram_tensor("ones", [128, NST * 2 * 64], bf16, kind=ikind).ap()
    seld = nc.dram_tensor("sel", [16, NST * NF], bf16, kind=ikind).ap()
    vd = nc.dram_tensor("vmat", [128, 32], f32, kind=ikind).ap()
    outd = nc.dram_tensor("out", [128, 32, T], f32, kind=okind).ap()
    benchd = None
    if bench:
        benchd = nc.dram_tensor("bench", [128, 32], f32,
                                kind="ExternalOutput").ap()

    with tile.TileContext(nc) as tc:
        with tc.tile_pool(name="consts", bufs=2) as consts, \
             tc.tile_pool(name="xp", bufs=1) as xp, \
             tc.tile_pool(name="x2p", bufs=10) as x2p, \
             tc.tile_pool(name="cmp", bufs=1) as cmp_, \
             tc.tile_pool(name="outp", bufs=5) as outp, \
             tc.tile_pool(name="ps_stats", bufs=2, space="PSUM") as ps_stats, \
             tc.tile_pool(name="ps_a", bufs=2, space="PSUM") as ps_a, \
             tc.tile_pool(name="ps_main", bufs=2, space="PSUM") as ps_main:

            vt = None
            for _rep in range(reps):
                # ---- constants ----
                wt = consts.tile([128, NST * NF], bf16, tag="w2", name="wt")
                nc.sync.dma_start(out=wt[:], in_=w2d[:])
                onest = consts.tile([128, NST * 2 * 64], bf16, tag="ones",
                                    name="onest")
                nc.sync.dma_start(out=onest[:], in_=onesd[:])
                selt = consts.tile([16, NST * NF], bf16, tag="sel",
                                   name="selt")
                nc.sync.dma_start(out=selt[:], in_=seld[:])
                vt = consts.tile([128, 32], f32, tag="vmat", name="vt")
                nc.sync.dma_start(out=vt[:], in_=vd[:])

                # ---- per group, software-pipelined ----
                # Phase A (stats+invstd) of group g is emitted before
                # phase B (scale/project/store) of group g-1, so the
                # in-order engines always have independent work to hide
                # the cross-engine invstd chain latency.
                def phase_a(g, res):
                    gi, c, K, s0 = g["gi"], g["c"], g["K"], g["s0"]
                    sts = g["sts"]
                    nst = len(sts)
                    inv_c = 1.0 / c

                    xt = xp.tile([128, nst * T], bf16, tag=f"X{gi}",
                                 name=f"xt{gi}", bufs=(1 if gi < 2 else 2))
                    res["xt"] = xt
                    if do_xdma:
                        for p0 in range(0, nst, 2):
                            p1 = min(p0 + 2, nst)
                            nc.sync.dma_start(
                                out=xt[:, p0 * T:p1 * T],
                                in_=xpd[:, (s0 + p0) * T:(s0 + p1) * T])
                    else:
                        nc.vector.memset(xt[:, 0:1], 0.0)

                    if mode == "dma":
                        for si, bands in enumerate(sts):
                            for (gband, _ig, off, _r0) in bands:
                                ot = outp.tile([128, T], f32, tag="O",
                                               name="ot")
                                nc.vector.memset(ot[:, 0:1], 0.0)
                                nc.sync.dma_start(out=outd[:, gband, :],
                                                  in_=ot[:])
                        return

                    # stats + per-chunk invstd (no group-wide barrier).
                    # varrb/rv/arbh are free-dim chunked: [K, ch*512 ...].
                    varrb = cmp_.tile([16, NCH * CHUNK], f32, tag="varrb",
                                      name="varrb")
                    rv = cmp_.tile([16, NCH * CHUNK], f32, tag="rv",
                                   name="rv")
                    arbh = cmp_.tile([16, NCH * CHUNK], bf16, tag="arbh",
                                     name="arbh", bufs=2)
                    res["arbh"] = arbh
                    if not do_dve:
                        nc.vector.memset(varrb[:, 0:1], 1.0)
                        nc.vector.memset(rv[:, 0:1], 1.0)
                    if not do_act:
                        nc.vector.memset(arbh[:, 0:1], 1.0)
                    sqs = {}
                    for ch in range(NCH):
                        cs = slice(ch * CHUNK, (ch + 1) * CHUNK)
                        stats = ps_stats.tile([64, CHUNK], f32, tag="stats",
                                              name="stats")
                        for si in range(nst):
                            st = s0 + si
                            xs = xt[:, si * T + ch * CHUNK:
                                    si * T + (ch + 1) * CHUNK]
                            if ch % 2 == 0:
                                sq = x2p.tile([128, 2 * CHUNK], bf16,
                                              tag="sq", name="sq")
                                sqs[si] = sq
                                if do_dve:
                                    xs2 = xt[:, si * T + ch * CHUNK:
                                             si * T + (ch + 2) * CHUNK]
                                    eng = nc.gpsimd if sqpool else nc.vector
                                    eng.tensor_mul(sq[:], xs2, xs2)
                                else:
                                    nc.vector.memset(sq[:, 0:1], 0.0)
                            sqv = sqs[si][:, (ch % 2) * CHUNK:
                                          (ch % 2 + 1) * CHUNK]
                            if do_mm:
                                nc.tensor.matmul(
                                    stats[:],
                                    onest[:, 2 * st * 64:(2 * st + 1) * 64],
                                    xs, start=(si == 0), stop=False,
                                    skip_group_check=True)
                                nc.tensor.matmul(
                                    stats[:],
                                    onest[:, (2 * st + 1) * 64:
                                          (2 * st + 2) * 64],
                                    sqv, start=False, stop=(si == nst - 1),
                                    skip_group_check=True)
                        if not do_mm:
                            nc.vector.memset(stats[:, 0:1], 1.0)
                        m2 = cmp_.tile([16, CHUNK], f32, tag="m2", name="m2")
                        if do_act:
                            nc.scalar.activation(m2[0:K, :], stats[0:K, :],
                                                 AF.Square, scale=inv_c)
                        else:
                            nc.vector.memset(m2[:, 0:1], 0.0)
                        if do_dve:
                            # var = E[X^2] - E[X]^2, then 1/var
                            nc.vector.scalar_tensor_tensor(
                                varrb[0:K, cs], stats[32:32 + K, :],
                                inv_c, m2[0:K, :], ALU.mult, ALU.subtract)
                            nc.vector.reciprocal(rv[0:K, cs], varrb[0:K, cs])
                        if do_act:
                            # invstd = sqrt(1/var), rounded to bf16
                            nc.scalar.activation(arbh[0:K, cs], rv[0:K, cs],
                                                 AF.Sqrt)
                        yield
                    yield

                def phase_b(g, xt, arbh):
                    gi, c, K, s0 = g["gi"], g["c"], g["K"], g["s0"]
                    sts = g["sts"]
                    for si, bands in enumerate(sts):
                        st = s0 + si
                        if atpair:
                            for h in range(2):
                                at = ps_a.tile([128, 2 * CHUNK], f32,
                                               tag="at", name="at", bufs=1)
                                if do_mm:
                                    for cc in range(2):
                                        ch = 2 * h + cc
                                        nc.tensor.matmul(
                                            at[:, cc * CHUNK:(cc + 1) * CHUNK],
                                            selt[0:K, st * NF:(st + 1) * NF],
                                            arbh[0:K, ch * CHUNK:
                                                 (ch + 1) * CHUNK],
                                            start=True, stop=True,
                                            skip_group_check=True)
                                elif do_dve:
                                    nc.vector.memset(at[:, 0:1], 1.0)
                                xs2 = xt[:, si * T + 2 * h * CHUNK:
                                         si * T + 2 * (h + 1) * CHUNK]
                                if do_dve:
                                    nc.vector.tensor_mul(xs2, xs2, at[:])
                        else:
                            for ch in range(NCH):
                                at = ps_a.tile([128, CHUNK], f32, tag="at",
                                               name="at")
                                if do_mm:
                                    nc.tensor.matmul(
                                        at[:],
                                        selt[0:K, st * NF:(st + 1) * NF],
                                        arbh[0:K, ch * CHUNK:(ch + 1) * CHUNK],
                                        start=True, stop=True)
                                elif do_dve:
                                    nc.vector.memset(at[:, 0:1], 1.0)
                                xs = xt[:, si * T + ch * CHUNK:
                                        si * T + (ch + 1) * CHUNK]
                                if do_dve:
                                    nc.vector.tensor_mul(xs, xs, at[:])
                        for (gband, _ig, off, _r0) in bands:
                            ot = outp.tile([128, T], f32, tag="O", name="ot")
                            nh, hw_ = (1, 4) if fulldrain else (2, 2)
                            for h in range(nh):
                                pm = ps_main.tile(
                                    [128, hw_ * CHUNK], f32, tag="pm",
                                    name="pm", bufs=(1 if fulldrain else 2))
                                if do_mm:
                                    for cc in range(hw_):
                                        ch = hw_ * h + cc
                                        nc.tensor.matmul(
                                            pm[:, cc * CHUNK:(cc + 1) * CHUNK],
                                            wt[off:off + c,
                                               st * NF:(st + 1) * NF],
                                            xt[off:off + c,
                                               si * T + ch * CHUNK:
                                               si * T + (ch + 1) * CHUNK],
                                            start=True, stop=True,
                                            skip_group_check=True,
                                            tile_position=(off, 0))
                                else:
                                    nc.vector.memset(pm[:, 0:1], 0.0)
                                if do_act:
                                    nc.scalar.activation(
                                        ot[:, h * hw_ * CHUNK:
                                           (h + 1) * hw_ * CHUNK],
                                        pm[:], AF.Identity,
                                        bias=vt[:, gband:gband + 1])
                                elif not do_out:
                                    nc.vector.memset(ot[:, 0:1], 0.0)
                            if do_out:
                                nc.sync.dma_start(out=outd[:, gband, :],
                                                  in_=ot[:])
                        yield

                def drain(gen):
                    for _ in gen:
                        pass

                def interleave(ga, gb):
                    # alternate emission; finish whichever runs long
                    while True:
                        a_live = b_live = False
                        if ga is not None:
                            try:
                                next(ga); a_live = True
                            except StopIteration:
                                ga = None
                        if gb is not None:
                            try:
                                next(gb); b_live = True
                            except StopIteration:
                                gb = None
                        if not (a_live or b_live):
                            return

                pending = None
                for g in groups:
                    res = {}
                    ga = phase_a(g, res)
                    if mode == "dma":
                        drain(ga)
                        continue
                    if not pipe:
                        drain(ga)
                        drain(phase_b(g, res["xt"], res["arbh"]))
                        continue
                    if pipe == "block":
                        drain(ga)
                        if pending is not None:
                            drain(phase_b(*pending))
                    else:
                        gb = (phase_b(*pending) if pending is not None
                              else None)
                        interleave(ga, gb)
                    pending = (g, res["xt"], res["arbh"])
                if pending is not None and mode != "dma":
                    drain(phase_b(*pending))
            if bench and benchd is not None and vt is not None:
                nc.sync.dma_start(out=benchd[:], in_=vt[:])
    return nc


def _split_excess_waits(nc, max_waits=1):
    """This walrus build rejects >1 semaphore wait on compute-instruction
    templates, while Tile freely attaches several. Hoist all but one wait
    onto standalone InstEventSemaphore instructions inserted just before,
    on the same engine — semantically identical (AND of ge-waits, engine
    stalls in program order)."""
    import concourse.mybir as mybir

    counter = 0
    for f in nc.m.functions:
        for blk in f.blocks:
            new_list = []
            changed = False
            for ins in blk.instructions:
                si = ins.sync_info
                ow = list(si.on_wait) if si is not None and si.on_wait else []
                if (
                    len(ow) > max_waits
                    and type(ins).__name__ != "InstEventSemaphore"
                    and all(w.wait_mode == "sem-ge-imm" for w in ow)
                ):
                    for w in ow[:-max_waits]:
                        ev = mybir.InstEventSemaphore(
                            name=f"evwait_split_{counter}", ins=[], outs=[]
                        )
                        counter += 1
                        ev.engine = ins.engine
                        ev.bass_nofuse = True
                        ev.debug = ins.debug
                        ev.sync_info = mybir.SyncInfo(on_wait=[w], on_update=[])
                        new_list.append(ev)
                    ins.sync_info = mybir.SyncInfo(
                        on_wait=ow[-max_waits:],
                        on_update=list(si.on_update) if si.on_update else [],
                    )
                    changed = True
                new_list.append(ins)
            if changed:
                blk.instructions = new_list
    return counter


def _get_nc(reps=1, bench=False, mode="full", pipe=True, fulldrain=False,
            atpair=True, sqpool=False):
    key = f"nc{reps}_{bench}_{mode}_{pipe}_{fulldrain}_{atpair}_{sqpool}"
    if key not in _cache:
        nc = _build_nc(reps, bench, mode, pipe, fulldrain, atpair, sqpool)
        _split_excess_waits(nc)
        _cache[key] = nc
    return _cache[key]


def _get_bench_nc_atpair(reps):
    return _get_nc(reps, bench=True, atpair=True)


def _get_bench_nc_sqpool(reps):
    return _get_nc(reps, bench=True, sqpool=True)


def _get_bench_nc_fulldrain(reps):
    return _get_nc(reps, bench=True, fulldrain=True)


def _get_bench_nc_noatpair(reps):
    return _get_nc(reps, bench=True, atpair=False)


def _get_bench_nc_block(reps):
    return _get_nc(reps, bench=True, pipe="block")


def _get_bench_nc_nopipe(reps):
    return _get_nc(reps, bench=True, pipe=False)


def _get_bench_nc(reps):
    return _get_nc(reps, bench=True)


def _get_bench_nc_dma(reps):
    return _get_nc(reps, bench=True, mode="dma")


def _get_bench_nc_compute(reps):
    return _get_nc(reps, bench=True, mode="compute")


def _bench_mode(mode):
    return lambda reps: _get_nc(reps, bench=True, mode=mode)


def make_imap(inputs):
    """Returns imap(core)->input dict, for the test harness's timing path."""
    consts = _precompute(inputs)
    xpack = _pack_x(np.asarray(inputs["x_real"], np.float32),
                    np.asarray(inputs["x_imag"], np.float32))

    def imap(b):
        return {
            "xpack": xpack[b], "w2": consts["w2"], "ones": consts["ones"],
            "sel": consts["sel"], "vmat": consts["vmat"],
        }
    return imap


def kernel(**inputs):
    from concourse.bass_utils import run_bass_kernel_spmd

    imap = make_imap(inputs)
    in_maps = [imap(b) for b in range(B)]
    nc = _get_nc()
    res = run_bass_kernel_spmd(nc, in_maps, list(range(B)))
    out = np.stack([res.results[b]["out"] for b in range(B)], axis=0)
    return out
